# revision 16
# baseline (speedup 1.0000x reference)
"""Distributed GAT (2-layer, BN between) on 8 Trainium2 NeuronCores.

Strategy:
- Partition destination nodes (and their incoming edges) across 8 cores.
- Phase A (sharded): z1 = x @ [W1|Wa1s|Wa1d] per node shard -> packed gather
  table rows [z1 f16 | as1 f32]; AllGather the table.
- L1 edge pass: edges sorted by dst, grouped in 128-edge blocks per 128-dst
  tile; per-edge rows fetched with dma_gather (4 SWDGE queues, one call per
  (tile, section), trailing -1 indices skipped via per-core num_idxs_reg);
  attention p = exp(leaky(as[src]+ad[dst])) built on-chip; scatter-add via
  selection-mask matmuls accumulating in PSUM; denominators likewise.
- maskE (scatter direction) generated on-chip: is_equal(dst-per-slot, iota).
  maskT (gather direction, for ad[dst] -> slot) loaded from HBM (needs dst on
  the free axis, which would require a partition broadcast on-chip).
- BatchNorm statistics via ones-matmul + AllReduce; affine folded to
  gamma', beta'.
- y/z2 computed in transposed layout (DMA-transpose) so BN affine+leaky are
  per-partition ops and z2 = y @ [W2|Wa2s|Wa2d] needs no on-chip transpose.
- L2 edge pass identical structure (1 head), reusing the same edge schedule,
  masks and gather indices.
"""
import sys
import types

sys.path.insert(0, "/opt/trn_rl_repo")

import numpy as np

# antenv.axon_hooks shim (needed only when tracing; harmless otherwise)
try:
    import antenv.axon_hooks  # noqa: F401
except Exception:
    try:
        import antenv

        _m = types.ModuleType("antenv.axon_hooks")
        _m._hook = None

        def _set(h):
            _m._hook = h

        def _get():
            return _m._hook

        _m.set_axon_ntff_profile_hook = _set
        _m.get_axon_ntff_profile_hook = _get
        sys.modules["antenv.axon_hooks"] = _m
        antenv.axon_hooks = _m
    except Exception:
        pass

import concourse.bacc as bacc
import concourse.mybir as mybir
import concourse.tile as tile
from concourse import bass_utils

F32 = mybir.dt.float32
F16 = mybir.dt.float16
I16 = mybir.dt.int16
I32 = mybir.dt.int32
OP = mybir.AluOpType
ACTF = mybir.ActivationFunctionType

N, E, F_IN, HID, HEADS, CLASSES = 50000, 800000, 128, 64, 4, 64
R = 8                      # cores
NS = N // R                # nodes per shard (6250)
NT = (NS + 127) // 128     # dst tiles per shard (49)
SECT = 25000               # gather-table section split (int16 index range)
HC = HEADS * HID           # 256
ROW1 = 384                 # halves per L1 table row: z(256) | as f32(8) | p | pad
ROW2 = 128                 # halves per L2 table row: z2(64) | as2 f32(2) | pad
W2C = CLASSES + 2          # 66
NEG_ATT = 0.2
NEG_ACT = 0.01
BN_EPS = 1e-5
MAXBLK = 8                 # blocks per dma_gather call
NQ = 4                     # SWDGE queues
MAXB = 24                  # mask/row tile block capacity


def _tile_nodes(t):
    return 128 if t < NT - 1 else NS - 128 * (NT - 1)


def _balance_perm(src, dst):
    """Assign nodes to (core, tile) bins so that per-bin edge counts are even
    in BOTH gather sections (the block count is set by the per-section max
    over cores). Side of a node keeps its original half (n < SECT), fixing
    per-dst section in-degrees (dA, dB) up front. Within each side, nodes are
    dealt round-robin: every round gives each open bin exactly one node, the
    heaviest nodes going to the bins with the lightest max-section load.
    Returns q: old node id -> new global id."""
    side = (np.arange(N) >= SECT).astype(np.int64)      # 0: cores 0-3
    sside = side[src]
    dA = np.bincount(dst[sside == 0], minlength=N)
    dB = np.bincount(dst[sside == 1], minlength=N)
    caps1 = np.array([_tile_nodes(t) for t in range(NT)], dtype=np.int64)
    base1 = np.concatenate([[0], np.cumsum(caps1[:-1])])
    q = np.empty(N, dtype=np.int64)
    for s in (0, 1):
        nodes = np.nonzero(side == s)[0]
        nodes = nodes[np.argsort(-(dA[nodes] + dB[nodes]), kind="stable")]
        nb_ = 4 * NT
        lA = np.zeros(nb_); lB = np.zeros(nb_)
        fill = np.zeros(nb_, dtype=np.int64)
        caps = np.tile(caps1, 4)
        base = (np.repeat(np.arange(4), NT) + 4 * s) * NS + np.tile(base1, 4)
        pos = 0
        while pos < len(nodes):
            open_b = np.nonzero(fill < caps)[0]
            k = min(len(open_b), len(nodes) - pos)
            taken = np.zeros(len(open_b), dtype=bool)
            for n in nodes[pos:pos + k]:
                cand = np.maximum(lA[open_b] + dA[n], lB[open_b] + dB[n])
                cand[taken] = 1e18
                j = int(np.argmin(cand))
                b = open_b[j]
                taken[j] = True
                q[n] = base[b] + fill[b]
                fill[b] += 1
                lA[b] += dA[n]; lB[b] += dB[n]
            pos += k
    return q


def plan(edge_index):
    """Host-side edge partitioning. Returns the (core-independent) schedule,
    per-core packed arrays, and the node permutation q (old id -> new id)."""
    ei = np.asarray(edge_index)
    # self-loops are handled locally on-chip (z/as/ad of a core's own nodes),
    # not gathered: this also removes their systematic per-side section skew
    src = ei[0].astype(np.int64)
    dst = ei[1].astype(np.int64)
    q = _balance_perm(src, dst)
    src, dst = q[src], q[dst]
    order = np.argsort(dst, kind="stable")
    src, dst = src[order], dst[order]

    # split each (core, tile) range, then sections by src < SECT
    core_of = dst // NS
    core_bounds = np.searchsorted(core_of, np.arange(R + 1))
    per = []  # per core: list over tiles of (srcA, dstA, srcB, dstB)
    for c in range(R):
        s0, s1 = core_bounds[c], core_bounds[c + 1]
        sc, dc = src[s0:s1], dst[s0:s1] - c * NS
        tb = np.searchsorted(dc // 128, np.arange(NT + 1))
        tiles = []
        for t in range(NT):
            st, dt_ = sc[tb[t]:tb[t + 1]], dc[tb[t]:tb[t + 1]] - t * 128
            a = st < SECT
            tiles.append((st[a], dt_[a], st[~a] - SECT, dt_[~a]))
        per.append(tiles)

    # common schedule: per tile, blocks per section = max over cores
    kA = [max(int(np.ceil(len(per[c][t][0]) / 128)) for c in range(R)) for t in range(NT)]
    kB = [max(int(np.ceil(len(per[c][t][2]) / 128)) for c in range(R)) for t in range(NT)]
    sched = []   # per tile: dict(blk0, nb, calls=[(sec, blk_off_in_tile, nb_call)])
    blk0 = 0
    ncalls = 0
    for t in range(NT):
        assert kA[t] + kB[t] <= MAXB, (t, kA[t], kB[t])
        calls = []
        off = 0
        for sec, k in ((0, kA[t]), (1, kB[t])):
            rem = k
            ncall = -(-k // MAXBLK) if k else 0
            while rem > 0:
                nb = -(-rem // ncall)
                calls.append((sec, off, nb, ncalls))
                ncalls += 1
                off += nb
                rem -= nb
                ncall -= 1
        sched.append({"t": t, "blk0": blk0, "nb": kA[t] + kB[t], "calls": calls,
                      "kA": kA[t], "kB": kB[t]})
        blk0 += kA[t] + kB[t]
    nblk = blk0

    # common per-call valid counts: max over cores, so the count can be a
    # static immediate (no per-core registers); cores with fewer edges pad
    # with dummy index 0 (dloc -1 -> masks zero the contribution)
    for t in range(NT):
        lens = [[len(per[c][t][0]) for c in range(R)], [len(per[c][t][2]) for c in range(R)]]
        for (sec, boff, nbc, ci) in sched[t]["calls"]:
            sec0 = 0 if sec == 0 else kA[t]
            lo, hi = (boff - sec0) * 128, (boff - sec0 + nbc) * 128
            sched[t].setdefault("nv", {})[ci] = max(
                max(0, min(l, hi) - lo) for l in lens[sec])

    # pack per-core arrays
    packs = []
    for c in range(R):
        idx = np.full((nblk * 128,), -1, dtype=np.int16)
        dloc = np.full((nblk * 128,), -1.0, dtype=np.float32)
        for t in range(NT):
            sA, dA, sB, dB = per[c][t]
            b0 = sched[t]["blk0"]
            for sec, (ss, dd), koff in ((0, (sA, dA), 0), (1, (sB, dB), kA[t])):
                o = (b0 + koff) * 128
                idx[o:o + len(ss)] = ss.astype(np.int16)
                dloc[o:o + len(ss)] = dd.astype(np.float32)
            # dummy-valid padding up to the common count of each call
            for (sec, boff, nbc, ci) in sched[t]["calls"]:
                nv = sched[t]["nv"][ci]
                sec0 = 0 if sec == 0 else kA[t]
                seclen = len(sA) if sec == 0 else len(sB)
                o = sched[t]["blk0"] * 128 + boff * 128
                have = max(0, min(seclen, (boff - sec0 + nbc) * 128) - (boff - sec0) * 128)
                if nv > have:
                    idx[o + have:o + nv] = 0
        # maskT [128 dloc, nblk, 128 p] fp16
        maskT = np.zeros((128, nblk, 128), dtype=np.float16)
        val = dloc >= 0
        j = np.nonzero(val)[0]
        maskT[dloc[j].astype(np.int64), j // 128, j % 128] = 1.0
        # idx wrapped: per call [16, ni/16] replicated to 128 partitions;
        # call col ranges == block col ranges (8 cols per block)
        w = idx.reshape(nblk * 8, 16).T          # [16, nblk*8]
        idx128 = np.tile(w, (8, 1))
        dstpp = dloc.reshape(nblk, 128).T.astype(np.float16).copy()  # [128, nblk]
        # per-slot exp-argument clamp: +10 (no-op) for real edges, -30 for
        # padding slots so p underflows to 0 and stale data can't blow up
        eclamp = np.where(dstpp >= 0, np.float16(10.0), np.float16(-30.0))
        packs.append({"idx": idx128, "dstpp": dstpp, "maskT": maskT,
                      "eclamp": eclamp})
    return sched, nblk, ncalls, packs, q


def host_inputs(x, edge_index, W1, a_src1, a_dst1, gamma, beta, W2, a_src2, a_dst2, b2):
    sched, nblk, ncalls, packs, q = plan(edge_index)
    inv = np.empty(N, dtype=np.int64)
    inv[q] = np.arange(N)
    x = np.asarray(x, dtype=np.float32)
    W1 = np.asarray(W1, dtype=np.float32)
    a_src1 = np.asarray(a_src1, dtype=np.float32)
    a_dst1 = np.asarray(a_dst1, dtype=np.float32)
    W2 = np.asarray(W2, dtype=np.float32)
    a_src2 = np.asarray(a_src2, dtype=np.float32)
    a_dst2 = np.asarray(a_dst2, dtype=np.float32)

    # Wa1s[f, h] = sum_c W1[f, h*HID + c] * a_src1[h, c]
    W1r = W1.reshape(F_IN, HEADS, HID)
    Wa1s = np.einsum("fhc,hc->fh", W1r, a_src1)
    Wa1d = np.einsum("fhc,hc->fh", W1r, a_dst1)
    W1ext = np.concatenate([W1, Wa1s, Wa1d], axis=1).astype(np.float32)  # [128, 264]

    Wa2s = W2 @ a_src2[0]        # [256]
    Wa2d = W2 @ a_dst2[0]
    W2ext = np.concatenate([W2, Wa2s[:, None], Wa2d[:, None]], axis=1).astype(np.float16)  # [256, 66]

    iota24 = np.tile(np.arange(128, dtype=np.float16)[None, None, :], (128, MAXB, 1))
    ones16 = np.ones((128, 1), dtype=np.float16)
    ones32 = np.ones((128, 1), dtype=np.float32)
    gb_in = np.concatenate([np.asarray(gamma, np.float32), np.asarray(beta, np.float32)])[None, :]  # [1,512]
    b2rep = np.tile(np.asarray(b2, np.float32)[None, :], (128, 1))  # [128, 64]

    ins = []
    for c in range(R):
        xT = np.ascontiguousarray(x[inv[c * NS:(c + 1) * NS]].T)  # [128, 6250]
        ins.append({
            "xT": xT,
            "W1ext": W1ext,
            "W2ext": W2ext,
            "iota24": iota24.reshape(128, MAXB * 128),
            "ones16": ones16,
            "ones32": ones32,
            "gb_in": gb_in,
            "b2rep": b2rep,
            "idx": packs[c]["idx"],
            "dstpp": packs[c]["dstpp"],
            "eclamp": packs[c]["eclamp"],
            "maskT": packs[c]["maskT"],
        })
    return sched, nblk, ncalls, ins, q


NSP = NT * 128  # padded shard rows (6272)


def build_program(sched, nblk, ncalls):
    nc = bacc.Bacc("TRN2", target_bir_lowering=False, debug=False,
                   num_devices=R, num_swdge_queues=NQ,
                   dynamic_dma_scratch_size=16384)
    xT_d = nc.dram_tensor("xT", [F_IN, NS], F32, kind="ExternalInput")
    W1e_d = nc.dram_tensor("W1ext", [F_IN, HC + 2 * HEADS], F32, kind="ExternalInput")
    W2e_d = nc.dram_tensor("W2ext", [HC, W2C], F16, kind="ExternalInput")
    iota24_d = nc.dram_tensor("iota24", [128, MAXB * 128], F16, kind="ExternalInput")
    ones16_d = nc.dram_tensor("ones16", [128, 1], F16, kind="ExternalInput")
    ones32_d = nc.dram_tensor("ones32", [128, 1], F32, kind="ExternalInput")
    gbin_d = nc.dram_tensor("gb_in", [1, 2 * HC], F32, kind="ExternalInput")
    b2rep_d = nc.dram_tensor("b2rep", [128, CLASSES], F32, kind="ExternalInput")
    idx_d = nc.dram_tensor("idx", [128, nblk * 8], I16, kind="ExternalInput")
    dstpp_d = nc.dram_tensor("dstpp", [128, nblk], F16, kind="ExternalInput")
    eclamp_d = nc.dram_tensor("eclamp", [128, nblk], F16, kind="ExternalInput")
    maskT_d = nc.dram_tensor("maskT", [128, nblk, 128], F16, kind="ExternalInput")
    out_d = nc.dram_tensor("out", [NS, CLASSES], F32, kind="ExternalOutput")

    with tile.TileContext(nc) as tc:
        import contextlib
        ctx = contextlib.ExitStack()
        with ctx:
            cons = ctx.enter_context(tc.tile_pool(name="cons", bufs=1))
            dram = ctx.enter_context(tc.tile_pool(name="dram", bufs=1, space="DRAM"))
            rowp = ctx.enter_context(tc.tile_pool(name="rowp", bufs=2))
            gp = ctx.enter_context(tc.tile_pool(name="gp", bufs=2))
            mp = ctx.enter_context(tc.tile_pool(name="mp", bufs=4))
            mtp = ctx.enter_context(tc.tile_pool(name="mtp", bufs=2))
            ep = ctx.enter_context(tc.tile_pool(name="ep", bufs=2))
            gop = ctx.enter_context(tc.tile_pool(name="gop", bufs=2))
            yp = ctx.enter_context(tc.tile_pool(name="yp", bufs=2))
            zlp = ctx.enter_context(tc.tile_pool(name="zlp", bufs=2))
            sfp = ctx.enter_context(tc.tile_pool(name="sfp", bufs=2))
            ps = ctx.enter_context(tc.tile_pool(name="ps", bufs=2, space="PSUM"))
            ps1 = ctx.enter_context(tc.tile_pool(name="ps1", bufs=1, space="PSUM"))
            bnp = ctx.enter_context(tc.tile_pool(name="bnp", bufs=1, space="PSUM"))

            # ---- constants into SBUF
            xT = cons.tile([F_IN, NS], F32)
            nc.sync.dma_start(out=xT[:], in_=xT_d[:, :])
            W1e = cons.tile([F_IN, HC + 2 * HEADS], F32)
            nc.sync.dma_start(out=W1e[:], in_=W1e_d[:, :])
            W2e0 = cons.tile([128, W2C], F16)
            nc.sync.dma_start(out=W2e0[:], in_=W2e_d[0:128, :])
            W2e1 = cons.tile([128, W2C], F16)
            nc.sync.dma_start(out=W2e1[:], in_=W2e_d[128:256, :])
            iota24 = cons.tile([128, MAXB, 128], F16)
            nc.sync.dma_start(out=iota24[:].rearrange("p a b -> p (a b)"),
                              in_=iota24_d[:, :])
            ones16 = cons.tile([128, 1], F16)
            nc.sync.dma_start(out=ones16[:], in_=ones16_d[:, :])
            ones32 = cons.tile([128, 1], F32)
            nc.sync.dma_start(out=ones32[:], in_=ones32_d[:, :])
            gbrow = cons.tile([1, 2 * HC], F32)
            nc.sync.dma_start(out=gbrow[:], in_=gbin_d[:, :])
            b2rep = cons.tile([128, CLASSES], F32)
            nc.sync.dma_start(out=b2rep[:], in_=b2rep_d[:, :])
            idx_t = cons.tile([128, nblk * 8], I16)
            nc.sync.dma_start(out=idx_t[:], in_=idx_d[:, :])
            dstpp = cons.tile([128, nblk], F16)
            nc.sync.dma_start(out=dstpp[:], in_=dstpp_d[:, :])
            eclamp = cons.tile([128, nblk], F16)
            nc.sync.dma_start(out=eclamp[:], in_=eclamp_d[:, :])
            ad_sh = cons.tile([128, NT, HEADS], F32)
            ad2_sh = cons.tile([128, NT, 1], F32)
            as_sh = cons.tile([128, NT, HEADS], F32)
            as2_sh = cons.tile([128, NT, 1], F32)
            nc.vector.memset(ad_sh[:], 0.0)
            nc.vector.memset(ad2_sh[:], 0.0)
            nc.vector.memset(as_sh[:], 0.0)
            nc.vector.memset(as2_sh[:], 0.0)

            # ---- internal DRAM
            tb1_sh = dram.tile([NS, ROW1], F16)
            tb1 = dram.tile([N, ROW1], F16, addr_space="Shared")
            g_sh = dram.tile([NSP, HC], F16)
            ar_in = dram.tile([1, 2 * HC], F32)
            ar_out = dram.tile([1, 2 * HC], F32, addr_space="Shared")
            gb_d = dram.tile([1, 2 * HC], F32)
            tb2_sh = dram.tile([NS, ROW2], F16)
            tb2 = dram.tile([N, ROW2], F16, addr_space="Shared")

            # zero-init the gather row buffers (slots skipped by the
            # negative-index gather must hold benign values, not NaN bits)
            for _ in range(4):
                rz = gp.tile([128, MAXB, ROW1], F16, tag="rows", bufs=4)
                nc.vector.memset(rz[:], 0.0)

            # ================= Phase A: z1/as1/ad1 tables =================
            for t in range(NT):
                nt = _tile_nodes(t)
                z1 = ps.tile([128, HC + 2 * HEADS], F32, tag="zps")
                nc.tensor.matmul(z1[0:nt, :], lhsT=xT[:, t * 128:t * 128 + nt],
                                 rhs=W1e[:], start=True, stop=True)
                row = rowp.tile([128, ROW1], F16, tag="row1")
                nc.vector.tensor_copy(out=row[0:nt, 0:HC], in_=z1[0:nt, 0:HC])
                nc.scalar.copy(
                    out=row[:].bitcast(F32)[0:nt, HC // 2:HC // 2 + HEADS],
                    in_=z1[0:nt, HC:HC + HEADS])
                nc.scalar.copy(out=ad_sh[0:nt, t, :],
                               in_=z1[0:nt, HC + HEADS:HC + 2 * HEADS])
                nc.scalar.copy(out=as_sh[0:nt, t, :], in_=z1[0:nt, HC:HC + HEADS])
                nc.sync.dma_start(out=tb1_sh[t * 128:t * 128 + nt, :], in_=row[0:nt, :])

            nc.gpsimd.collective_compute(
                "AllGather", OP.bypass, replica_groups=[list(range(R))],
                ins=[tb1_sh.opt()], outs=[tb1.opt()])

            # ================= L1 edge pass =================
            ad16_all = cons.tile([128, NT, HEADS], F16)
            nc.scalar.copy(out=ad16_all[:], in_=ad_sh[:])
            bn = bnp.tile([1, 2 * HC], F32, tag="bn")
            for ti, S in enumerate(sched):
                t, b0, nb = S["t"], S["blk0"], S["nb"]
                nt = _tile_nodes(t)
                mT = mtp.tile([128, MAXB, 128], F16, tag="mT", bufs=4)
                nc.sync.dma_start(out=mT[:, 0:nb, :], in_=maskT_d[:, b0:b0 + nb, :])
                mE = mtp.tile([128, MAXB, 128], F16, tag="mE", bufs=4)
                nc.vector.tensor_tensor(
                    out=mE[:, 0:nb, :],
                    in0=dstpp[:, b0:b0 + nb].to_broadcast([128, nb, 128]),
                    in1=iota24[:, 0:nb, :],
                    op=OP.is_equal)
                out_ps = ps.tile([128, HC + HEADS], F32, tag="outps", bufs=4)
                ad_ps = ps1.tile([128, MAXB * HEADS], F32, tag="adps")
                rows = gp.tile([128, MAXB, ROW1], F16, tag="rows", bufs=4)
                for (sec, boff, nbc, ci) in S["calls"]:
                    ni = nbc * 128
                    base = sec * SECT
                    nc.gpsimd.dma_gather(
                        rows[:, boff:boff + nbc, :], tb1[base:base + SECT, :],
                        idx_t[:, (b0 + boff) * 8:(b0 + boff) * 8 + nbc * 8],
                        ni, S["nv"][ci], ROW1, queue_num=ci % NQ)
                for b in range(nb):
                    nc.tensor.matmul(ad_ps[:, b * HEADS:(b + 1) * HEADS],
                                     lhsT=mT[:, b, :], rhs=ad16_all[:, t, :],
                                     start=True, stop=True)
                e_t = ep.tile([128, MAXB, HEADS], F32, tag="e")
                nc.vector.tensor_tensor(
                    out=e_t[:, 0:nb, :],
                    in0=rows[:].bitcast(F32)[:, 0:nb, HC // 2:HC // 2 + HEADS],
                    in1=ad_ps[:, 0:nb * HEADS].rearrange("p (b h) -> p b h", h=HEADS),
                    op=OP.add)
                nc.scalar.activation(e_t[:, 0:nb, :], e_t[:, 0:nb, :], ACTF.Lrelu,
                                      alpha=NEG_ATT)
                nc.vector.tensor_tensor(out=e_t[:, 0:nb, :], in0=e_t[:, 0:nb, :],
                                        in1=eclamp[:, b0:b0 + nb].to_broadcast([128, nb, HEADS]),
                                        op=OP.min)
                nc.scalar.activation(rows[:, 0:nb, HC:HC + HEADS],
                                     e_t[:, 0:nb, :], ACTF.Exp)
                nc.vector.tensor_tensor(
                    out=rows[:, 0:nb, 0:HC].rearrange("p b (h c) -> p b h c", h=HEADS),
                    in0=rows[:, 0:nb, 0:HC].rearrange("p b (h c) -> p b h c", h=HEADS),
                    in1=rows[:, 0:nb, HC:HC + HEADS].to_broadcast([128, nb, HEADS, HID]),
                    op=OP.mult)
                for b in range(nb):
                    nc.tensor.matmul(out_ps[:, :], lhsT=mE[:, b, :],
                                     rhs=rows[:, b, 0:HC + HEADS],
                                     start=(b == 0), stop=(b == nb - 1))
                z1loc = zlp.tile([128, HC], F16, tag="z1loc")
                nc.sync.dma_start(out=z1loc[0:nt, :], in_=tb1_sh[t * 128:t * 128 + nt, 0:HC])
                es = ep.tile([128, HEADS], F32, tag="es")
                nc.vector.tensor_tensor(out=es[:], in0=as_sh[:, t, :],
                                        in1=ad_sh[:, t, :], op=OP.add)
                nc.scalar.activation(es[:], es[:], ACTF.Lrelu, alpha=NEG_ATT)
                psf = ep.tile([128, HEADS], F32, tag="psf")
                nc.scalar.activation(psf[:], es[:], ACTF.Exp)
                dtm = ep.tile([128, HEADS], F32, tag="dtm")
                nc.vector.tensor_tensor(out=dtm[:], in0=out_ps[:, HC:HC + HEADS],
                                        in1=psf[:], op=OP.add)
                dre = ep.tile([128, HEADS], F32, tag="dre")
                nc.vector.reciprocal(out=dre[:], in_=dtm[:])
                sfw = sfp.tile([128, HC], F32, tag="sfw")
                nc.vector.tensor_tensor(
                    out=sfw[:].rearrange("p (h c) -> p h c", h=HEADS),
                    in0=z1loc[:].rearrange("p (h c) -> p h c", h=HEADS),
                    in1=psf[:].to_broadcast([128, HEADS, HID]),
                    op=OP.mult)
                nc.vector.tensor_tensor(out=sfw[:], in0=sfw[:],
                                        in1=out_ps[:, 0:HC], op=OP.add)
                gsq = gop.tile([128, 2 * HC], F16, tag="gsq")
                nc.vector.tensor_tensor(
                    out=gsq[:, 0:HC].rearrange("p (h c) -> p h c", h=HEADS),
                    in0=sfw[:].rearrange("p (h c) -> p h c", h=HEADS),
                    in1=dre[:].to_broadcast([128, HEADS, HID]),
                    op=OP.mult)
                nc.vector.tensor_tensor(out=gsq[0:nt, HC:2 * HC], in0=gsq[0:nt, 0:HC],
                                        in1=gsq[0:nt, 0:HC], op=OP.mult)
                nc.tensor.matmul(bn[:, :], lhsT=ones16[0:nt, :], rhs=gsq[0:nt, :],
                                 start=(ti == 0), stop=(ti == NT - 1))
                nc.sync.dma_start(out=g_sh[t * 128:t * 128 + nt, :], in_=gsq[0:nt, 0:HC])

            # zero the padded tail rows of g_sh
            zr = gop.tile([128, 2 * HC], F16, tag="gsq")
            nc.vector.memset(zr[:], 0.0)
            if NSP > NS:
                nc.sync.dma_start(out=g_sh[NS:NSP, :], in_=zr[0:NSP - NS, 0:HC])

            # ================= BN stats: AllReduce + affine =================
            bnst = cons.tile([1, 2 * HC], F32)
            nc.vector.tensor_copy(out=bnst[:], in_=bn[:, :])
            nc.sync.dma_start(out=ar_in[:], in_=bnst[:])
            nc.gpsimd.collective_compute(
                "AllReduce", OP.add, replica_groups=[list(range(R))],
                ins=[ar_in.opt()], outs=[ar_out.opt()])
            st = cons.tile([1, 2 * HC], F32)
            nc.sync.dma_start(out=st[:], in_=ar_out[:])
            mu = cons.tile([1, HC], F32)
            nc.vector.tensor_scalar(out=mu[:], in0=st[:, 0:HC], scalar1=1.0 / N,
                                    scalar2=None, op0=OP.mult)
            var = cons.tile([1, HC], F32)
            nc.vector.tensor_scalar(out=var[:], in0=st[:, HC:2 * HC], scalar1=1.0 / N,
                                    scalar2=None, op0=OP.mult)
            musq = cons.tile([1, HC], F32)
            nc.vector.tensor_tensor(out=musq[:], in0=mu[:], in1=mu[:], op=OP.mult)
            nc.vector.tensor_tensor(out=var[:], in0=var[:], in1=musq[:], op=OP.subtract)
            nc.vector.tensor_scalar(out=var[:], in0=var[:], scalar1=BN_EPS, scalar2=None,
                                    op0=OP.add)
            rv = cons.tile([1, HC], F32)
            nc.vector.reciprocal(out=rv[:], in_=var[:])
            rs = cons.tile([1, HC], F32)
            nc.scalar.activation(rs[:], rv[:], ACTF.Sqrt)
            gp_ = cons.tile([1, HC], F32)
            nc.vector.tensor_tensor(out=gp_[:], in0=rs[:], in1=gbrow[:, 0:HC], op=OP.mult)
            bp_ = cons.tile([1, HC], F32)
            nc.vector.tensor_tensor(out=bp_[:], in0=gp_[:], in1=mu[:], op=OP.mult)
            nc.vector.tensor_tensor(out=bp_[:], in0=gbrow[:, HC:2 * HC], in1=bp_[:],
                                    op=OP.subtract)
            nc.sync.dma_start(out=gb_d[0:1, 0:HC], in_=gp_[:])
            nc.sync.dma_start(out=gb_d[0:1, HC:2 * HC], in_=bp_[:])

            # read gamma'/beta' transposed: [128,1] per feature half
            gb_cols = gb_d.rearrange("a (f x) -> (a f) x", x=1)
            gpp = [cons.tile([128, 1], F32, name=f"gpp{i}") for i in range(2)]
            bpp = [cons.tile([128, 1], F32, name=f"bpp{i}") for i in range(2)]
            for h in range(2):
                nc.sync.dma_start(out=gpp[h][:], in_=gb_cols[h * 128:(h + 1) * 128, :])
                nc.sync.dma_start(out=bpp[h][:], in_=gb_cols[HC + h * 128:HC + (h + 1) * 128, :])

            # ================= P4: y (transposed) and z2 table =================
            chunks = [(i * 512, 512) for i in range(NSP // 512)]
            if NSP % 512:
                chunks.append((NSP - NSP % 512, NSP % 512))
            for (n0, w) in chunks:
                yT = []
                for h in range(2):
                    gT = yp.tile([128, 512], F16, tag=f"gT{h}")
                    nc.sync.dma_start(out=gT[:, 0:w],
                                      in_=g_sh[n0:n0 + w, h * 128:(h + 1) * 128],
                                      transpose=True)
                    nc.vector.tensor_scalar(out=gT[:, 0:w], in0=gT[:, 0:w],
                                            scalar1=gpp[h][:, :], scalar2=bpp[h][:, :],
                                            op0=OP.mult, op1=OP.add)
                    nc.scalar.activation(gT[:, 0:w], gT[:, 0:w], ACTF.Lrelu,
                                          alpha=NEG_ACT)
                    yT.append(gT)
                for i in range(w // 128):
                    t = (n0 + i * 128) // 128
                    nt = _tile_nodes(t) if t < NT else 0
                    if nt == 0:
                        continue
                    z2 = ps.tile([128, W2C], F32, tag="zps")
                    nc.tensor.matmul(z2[:, :], lhsT=yT[0][:, i * 128:(i + 1) * 128],
                                     rhs=W2e0[:], start=True, stop=False)
                    nc.tensor.matmul(z2[:, :], lhsT=yT[1][:, i * 128:(i + 1) * 128],
                                     rhs=W2e1[:], start=False, stop=True)
                    row2 = rowp.tile([128, ROW2], F16, tag="row2")
                    nc.scalar.copy(out=row2[0:nt, 0:CLASSES], in_=z2[0:nt, 0:CLASSES])
                    nc.scalar.copy(
                        out=row2[:].bitcast(F32)[0:nt, CLASSES // 2:CLASSES // 2 + 1],
                        in_=z2[0:nt, CLASSES:CLASSES + 1])
                    nc.scalar.copy(out=ad2_sh[0:nt, t, :],
                                   in_=z2[0:nt, CLASSES + 1:CLASSES + 2])
                    nc.scalar.copy(out=as2_sh[0:nt, t, :],
                                   in_=z2[0:nt, CLASSES:CLASSES + 1])
                    nc.sync.dma_start(out=tb2_sh[t * 128:t * 128 + nt, :], in_=row2[0:nt, :])

            nc.gpsimd.collective_compute(
                "AllGather", OP.bypass, replica_groups=[list(range(R))],
                ins=[tb2_sh.opt()], outs=[tb2.opt()])

            # ================= L2 edge pass =================
            # re-zero the gather buffers: stale L1-format bytes reinterpreted
            # at L2 offsets can alias to f16 NaN patterns (0 * NaN = NaN)
            for _ in range(4):
                rz2 = gp.tile([128, MAXB, ROW1], F16, tag="rows", bufs=4)
                nc.vector.memset(rz2[:], 0.0)
            ad2_16 = cons.tile([128, NT, 1], F16)
            nc.scalar.copy(out=ad2_16[:], in_=ad2_sh[:])
            for S in sched:
                t, b0, nb = S["t"], S["blk0"], S["nb"]
                nt = _tile_nodes(t)
                mT = mtp.tile([128, MAXB, 128], F16, tag="mT", bufs=4)
                nc.sync.dma_start(out=mT[:, 0:nb, :], in_=maskT_d[:, b0:b0 + nb, :])
                mE = mtp.tile([128, MAXB, 128], F16, tag="mE", bufs=4)
                nc.vector.tensor_tensor(
                    out=mE[:, 0:nb, :],
                    in0=dstpp[:, b0:b0 + nb].to_broadcast([128, nb, 128]),
                    in1=iota24[:, 0:nb, :],
                    op=OP.is_equal)
                out_ps = ps.tile([128, HC + HEADS], F32, tag="outps", bufs=4)
                ad_ps = ps1.tile([128, MAXB * HEADS], F32, tag="adps")
                rows = gp.tile([128, MAXB, ROW2], F16, tag="rows", bufs=4)
                for (sec, boff, nbc, ci) in S["calls"]:
                    ni = nbc * 128
                    base = sec * SECT
                    nc.gpsimd.dma_gather(
                        rows[:, boff:boff + nbc, :], tb2[base:base + SECT, :],
                        idx_t[:, (b0 + boff) * 8:(b0 + boff) * 8 + nbc * 8],
                        ni, S["nv"][ci], ROW2, queue_num=ci % NQ)
                for b in range(nb):
                    nc.tensor.matmul(ad_ps[:, b:b + 1],
                                     lhsT=mT[:, b, :], rhs=ad2_16[:, t, :],
                                     start=True, stop=True)
                e_t = ep.tile([128, MAXB, HEADS], F32, tag="e")
                nc.vector.tensor_tensor(
                    out=e_t[:, 0:nb, 0:1],
                    in0=rows[:].bitcast(F32)[:, 0:nb, CLASSES // 2:CLASSES // 2 + 1],
                    in1=ad_ps[:, 0:nb].rearrange("p (b h) -> p b h", h=1),
                    op=OP.add)
                e2_t = ep.tile([128, MAXB, HEADS], F32, tag="e2")
                nc.vector.tensor_scalar(out=e2_t[:, 0:nb, 0:1], in0=e_t[:, 0:nb, 0:1],
                                        scalar1=NEG_ATT, scalar2=None, op0=OP.mult)
                nc.vector.tensor_tensor(out=e_t[:, 0:nb, 0:1], in0=e_t[:, 0:nb, 0:1],
                                        in1=e2_t[:, 0:nb, 0:1], op=OP.max)
                nc.vector.tensor_tensor(out=e_t[:, 0:nb, 0:1], in0=e_t[:, 0:nb, 0:1],
                                        in1=eclamp[:, b0:b0 + nb].to_broadcast([128, nb, 1]),
                                        op=OP.min)
                nc.scalar.activation(rows[:, 0:nb, CLASSES:CLASSES + 1],
                                     e_t[:, 0:nb, 0:1], ACTF.Exp)
                nc.vector.tensor_tensor(
                    out=rows[:, 0:nb, 0:CLASSES],
                    in0=rows[:, 0:nb, 0:CLASSES],
                    in1=rows[:, 0:nb, CLASSES:CLASSES + 1].to_broadcast([128, nb, CLASSES]),
                    op=OP.mult)
                for b in range(nb):
                    nc.tensor.matmul(out_ps[:, 0:CLASSES + 1], lhsT=mE[:, b, :],
                                     rhs=rows[:, b, 0:CLASSES + 1],
                                     start=(b == 0), stop=(b == nb - 1))
                z2loc = zlp.tile([128, CLASSES], F16, tag="z2loc")
                nc.sync.dma_start(out=z2loc[0:nt, :],
                                  in_=tb2_sh[t * 128:t * 128 + nt, 0:CLASSES])
                es = ep.tile([128, HEADS], F32, tag="es")
                nc.vector.tensor_tensor(out=es[:, 0:1], in0=as2_sh[:, t, :],
                                        in1=ad2_sh[:, t, :], op=OP.add)
                nc.scalar.activation(es[:, 0:1], es[:, 0:1], ACTF.Lrelu, alpha=NEG_ATT)
                psf = ep.tile([128, HEADS], F32, tag="psf")
                nc.scalar.activation(psf[:, 0:1], es[:, 0:1], ACTF.Exp)
                dtm = ep.tile([128, HEADS], F32, tag="dtm")
                nc.vector.tensor_tensor(out=dtm[:, 0:1], in0=out_ps[:, CLASSES:CLASSES + 1],
                                        in1=psf[:, 0:1], op=OP.add)
                dre = ep.tile([128, 1], F32, tag="dre2")
                nc.vector.reciprocal(out=dre[:], in_=dtm[:, 0:1])
                sfw = sfp.tile([128, HC], F32, tag="sfw")
                nc.vector.tensor_tensor(out=sfw[:, 0:CLASSES], in0=z2loc[:],
                                        in1=psf[:, 0:1].to_broadcast([128, CLASSES]),
                                        op=OP.mult)
                nc.vector.tensor_tensor(out=sfw[:, 0:CLASSES], in0=sfw[:, 0:CLASSES],
                                        in1=out_ps[:, 0:CLASSES], op=OP.add)
                o32 = gop.tile([128, CLASSES], F32, tag="o32")
                nc.vector.tensor_scalar(out=o32[:], in0=sfw[:, 0:CLASSES], scalar1=dre[:, :],
                                        scalar2=None, op0=OP.mult)
                nc.vector.tensor_tensor(out=o32[:], in0=o32[:], in1=b2rep[:], op=OP.add)
                nc.sync.dma_start(out=out_d[t * 128:t * 128 + nt, :], in_=o32[0:nt, :])
    nc.compile()
    return nc


def kernel(x, edge_index, W1, a_src1, a_dst1, b1, gamma, beta, W2, a_src2, a_dst2, b2):
    # b1 cancels inside BatchNorm (constant per-channel shift) -> unused.
    sched, nblk, ncalls, ins, q = host_inputs(np.asarray(x), np.asarray(edge_index),
                                              W1, a_src1, a_dst1, gamma, beta,
                                              W2, a_src2, a_dst2, b2)
    nc = build_program(sched, nblk, ncalls)
    res = bass_utils.run_bass_kernel_spmd(nc, ins, core_ids=list(range(R)))
    out = np.concatenate([res.results[c]["out"] for c in range(R)], axis=0)
    return out[q].astype(np.float32)


# revision 20
# speedup vs baseline: 1.1383x; 1.1383x over previous
"""Distributed GAT (2-layer, BN between) on 8 Trainium2 NeuronCores.

Strategy:
- Partition destination nodes (and their incoming edges) across 8 cores.
- Phase A (sharded): z1 = x @ [W1|Wa1s|Wa1d] per node shard -> packed gather
  table rows [z1 f16 | as1 f32]; AllGather the table.
- L1 edge pass: edges sorted by dst, grouped in 128-edge blocks per 128-dst
  tile; per-edge rows fetched with dma_gather (4 SWDGE queues, one call per
  (tile, section), trailing -1 indices skipped via per-core num_idxs_reg);
  attention p = exp(leaky(as[src]+ad[dst])) built on-chip; scatter-add via
  selection-mask matmuls accumulating in PSUM; denominators likewise.
- maskE (scatter direction) generated on-chip: is_equal(dst-per-slot, iota).
  maskT (gather direction, for ad[dst] -> slot) loaded from HBM (needs dst on
  the free axis, which would require a partition broadcast on-chip).
- BatchNorm statistics via ones-matmul + AllReduce; affine folded to
  gamma', beta'.
- y/z2 computed in transposed layout (DMA-transpose) so BN affine+leaky are
  per-partition ops and z2 = y @ [W2|Wa2s|Wa2d] needs no on-chip transpose.
- L2 edge pass identical structure (1 head), reusing the same edge schedule,
  masks and gather indices.
"""
import sys
import types

sys.path.insert(0, "/opt/trn_rl_repo")

import numpy as np

# antenv.axon_hooks shim (needed only when tracing; harmless otherwise)
try:
    import antenv.axon_hooks  # noqa: F401
except Exception:
    try:
        import antenv

        _m = types.ModuleType("antenv.axon_hooks")
        _m._hook = None

        def _set(h):
            _m._hook = h

        def _get():
            return _m._hook

        _m.set_axon_ntff_profile_hook = _set
        _m.get_axon_ntff_profile_hook = _get
        sys.modules["antenv.axon_hooks"] = _m
        antenv.axon_hooks = _m
    except Exception:
        pass

import concourse.bacc as bacc
import concourse.mybir as mybir
import concourse.tile as tile
from concourse import bass_utils

F32 = mybir.dt.float32
F16 = mybir.dt.float16
I16 = mybir.dt.int16
I32 = mybir.dt.int32
OP = mybir.AluOpType
ACTF = mybir.ActivationFunctionType

N, E, F_IN, HID, HEADS, CLASSES = 50000, 800000, 128, 64, 4, 64
R = 8                      # cores
NS = N // R                # nodes per shard (6250)
NT = (NS + 127) // 128     # dst tiles per shard (49)
SECT = 25000               # gather-table section split (int16 index range)
HC = HEADS * HID           # 256
ROW1 = 384                 # halves per L1 table row: z(256) | as f32(8) | p | pad
ROW2 = 128                 # halves per L2 table row: z2(64) | as2 f32(2) | pad
W2C = CLASSES + 2          # 66
NEG_ATT = 0.2
NEG_ACT = 0.01
BN_EPS = 1e-5
MAXBLK = 8                 # blocks per dma_gather call
NQ = 4                     # SWDGE queues
MAXB = 24                  # mask/row tile block capacity


def _tile_nodes(t):
    return 128 if t < NT - 1 else NS - 128 * (NT - 1)


def _balance_perm(src, dst):
    """Assign nodes to (core, tile) bins so that per-bin edge counts are even
    in BOTH gather sections (the block count is set by the per-section max
    over cores). Side of a node keeps its original half (n < SECT), fixing
    per-dst section in-degrees (dA, dB) up front. Within each side, nodes are
    dealt round-robin: every round gives each open bin exactly one node, the
    heaviest nodes going to the bins with the lightest max-section load.
    Returns q: old node id -> new global id."""
    side = (np.arange(N) >= SECT).astype(np.int64)      # 0: cores 0-3
    sside = side[src]
    dA = np.bincount(dst[sside == 0], minlength=N)
    dB = np.bincount(dst[sside == 1], minlength=N)
    caps1 = np.array([_tile_nodes(t) for t in range(NT)], dtype=np.int64)
    base1 = np.concatenate([[0], np.cumsum(caps1[:-1])])
    q = np.empty(N, dtype=np.int64)
    for s in (0, 1):
        nodes = np.nonzero(side == s)[0]
        nodes = nodes[np.argsort(-(dA[nodes] + dB[nodes]), kind="stable")]
        nb_ = 4 * NT
        lA = np.zeros(nb_); lB = np.zeros(nb_)
        fill = np.zeros(nb_, dtype=np.int64)
        caps = np.tile(caps1, 4)
        base = (np.repeat(np.arange(4), NT) + 4 * s) * NS + np.tile(base1, 4)
        pos = 0
        while pos < len(nodes):
            open_b = np.nonzero(fill < caps)[0]
            k = min(len(open_b), len(nodes) - pos)
            taken = np.zeros(len(open_b), dtype=bool)
            for n in nodes[pos:pos + k]:
                cand = np.maximum(lA[open_b] + dA[n], lB[open_b] + dB[n])
                cand[taken] = 1e18
                j = int(np.argmin(cand))
                b = open_b[j]
                taken[j] = True
                q[n] = base[b] + fill[b]
                fill[b] += 1
                lA[b] += dA[n]; lB[b] += dB[n]
            pos += k
    return q


def plan(edge_index):
    """Host-side edge partitioning. Returns the (core-independent) schedule,
    per-core packed arrays, and the node permutation q (old id -> new id)."""
    ei = np.asarray(edge_index)
    # self-loops are handled locally on-chip (z/as/ad of a core's own nodes),
    # not gathered: this also removes their systematic per-side section skew
    src = ei[0].astype(np.int64)
    dst = ei[1].astype(np.int64)
    q = _balance_perm(src, dst)
    src, dst = q[src], q[dst]
    order = np.argsort(dst, kind="stable")
    src, dst = src[order], dst[order]

    # split each (core, tile) range, then sections by src < SECT
    core_of = dst // NS
    core_bounds = np.searchsorted(core_of, np.arange(R + 1))
    per = []  # per core: list over tiles of (srcA, dstA, srcB, dstB)
    for c in range(R):
        s0, s1 = core_bounds[c], core_bounds[c + 1]
        sc, dc = src[s0:s1], dst[s0:s1] - c * NS
        tb = np.searchsorted(dc // 128, np.arange(NT + 1))
        tiles = []
        for t in range(NT):
            st, dt_ = sc[tb[t]:tb[t + 1]], dc[tb[t]:tb[t + 1]] - t * 128
            a = st < SECT
            tiles.append((st[a], dt_[a], st[~a] - SECT, dt_[~a]))
        per.append(tiles)

    # common schedule: per tile, blocks per section = max over cores
    kA = [max(int(np.ceil(len(per[c][t][0]) / 128)) for c in range(R)) for t in range(NT)]
    kB = [max(int(np.ceil(len(per[c][t][2]) / 128)) for c in range(R)) for t in range(NT)]
    sched = []   # per tile: dict(blk0, nb, calls=[(sec, blk_off_in_tile, nb_call)])
    blk0 = 0
    ncalls = 0
    for t in range(NT):
        assert kA[t] + kB[t] <= MAXB, (t, kA[t], kB[t])
        calls = []
        off = 0
        for sec, k in ((0, kA[t]), (1, kB[t])):
            rem = k
            ncall = -(-k // MAXBLK) if k else 0
            while rem > 0:
                nb = -(-rem // ncall)
                calls.append((sec, off, nb, ncalls))
                ncalls += 1
                off += nb
                rem -= nb
                ncall -= 1
        sched.append({"t": t, "blk0": blk0, "nb": kA[t] + kB[t], "calls": calls,
                      "kA": kA[t], "kB": kB[t]})
        blk0 += kA[t] + kB[t]
    nblk = blk0

    # common per-call valid counts: max over cores, so the count can be a
    # static immediate (no per-core registers); cores with fewer edges pad
    # with dummy index 0 (dloc -1 -> masks zero the contribution)
    for t in range(NT):
        lens = [[len(per[c][t][0]) for c in range(R)], [len(per[c][t][2]) for c in range(R)]]
        for (sec, boff, nbc, ci) in sched[t]["calls"]:
            sec0 = 0 if sec == 0 else kA[t]
            lo, hi = (boff - sec0) * 128, (boff - sec0 + nbc) * 128
            sched[t].setdefault("nv", {})[ci] = max(
                max(0, min(l, hi) - lo) for l in lens[sec])

    # pack per-core arrays
    packs = []
    for c in range(R):
        idx = np.full((nblk * 128,), -1, dtype=np.int16)
        dloc = np.full((nblk * 128,), -1.0, dtype=np.float32)
        for t in range(NT):
            sA, dA, sB, dB = per[c][t]
            b0 = sched[t]["blk0"]
            for sec, (ss, dd), koff in ((0, (sA, dA), 0), (1, (sB, dB), kA[t])):
                o = (b0 + koff) * 128
                idx[o:o + len(ss)] = ss.astype(np.int16)
                dloc[o:o + len(ss)] = dd.astype(np.float32)
            # dummy-valid padding up to the common count of each call
            for (sec, boff, nbc, ci) in sched[t]["calls"]:
                nv = sched[t]["nv"][ci]
                sec0 = 0 if sec == 0 else kA[t]
                seclen = len(sA) if sec == 0 else len(sB)
                o = sched[t]["blk0"] * 128 + boff * 128
                have = max(0, min(seclen, (boff - sec0 + nbc) * 128) - (boff - sec0) * 128)
                if nv > have:
                    idx[o + have:o + nv] = 0
        # maskT [128 dloc, nblk, 128 p] fp16
        maskT = np.zeros((128, nblk, 128), dtype=np.float16)
        val = dloc >= 0
        j = np.nonzero(val)[0]
        maskT[dloc[j].astype(np.int64), j // 128, j % 128] = 1.0
        # idx wrapped: per call [16, ni/16] replicated to 128 partitions;
        # call col ranges == block col ranges (8 cols per block)
        w = idx.reshape(nblk * 8, 16).T          # [16, nblk*8]
        idx128 = np.tile(w, (8, 1))
        dstpp = dloc.reshape(nblk, 128).T.astype(np.float16).copy()  # [128, nblk]
        # per-slot exp-argument clamp: +10 (no-op) for real edges, -30 for
        # padding slots so p underflows to 0 and stale data can't blow up
        eclamp = np.where(dstpp >= 0, np.float16(10.0), np.float16(-30.0))
        packs.append({"idx": idx128, "dstpp": dstpp, "maskT": maskT,
                      "eclamp": eclamp})
    return sched, nblk, ncalls, packs, q


def host_inputs(x, edge_index, W1, a_src1, a_dst1, gamma, beta, W2, a_src2, a_dst2, b2):
    sched, nblk, ncalls, packs, q = plan(edge_index)
    inv = np.empty(N, dtype=np.int64)
    inv[q] = np.arange(N)
    x = np.asarray(x, dtype=np.float32)
    W1 = np.asarray(W1, dtype=np.float32)
    a_src1 = np.asarray(a_src1, dtype=np.float32)
    a_dst1 = np.asarray(a_dst1, dtype=np.float32)
    W2 = np.asarray(W2, dtype=np.float32)
    a_src2 = np.asarray(a_src2, dtype=np.float32)
    a_dst2 = np.asarray(a_dst2, dtype=np.float32)

    # Wa1s[f, h] = sum_c W1[f, h*HID + c] * a_src1[h, c]
    W1r = W1.reshape(F_IN, HEADS, HID)
    Wa1s = np.einsum("fhc,hc->fh", W1r, a_src1)
    Wa1d = np.einsum("fhc,hc->fh", W1r, a_dst1)
    W1ext = np.concatenate([W1, Wa1s, Wa1d], axis=1).astype(np.float32)  # [128, 264]

    Wa2s = W2 @ a_src2[0]        # [256]
    Wa2d = W2 @ a_dst2[0]
    W2ext = np.concatenate([W2, Wa2s[:, None], Wa2d[:, None]], axis=1).astype(np.float16)  # [256, 66]

    iota24 = np.tile(np.arange(128, dtype=np.float16)[None, None, :], (128, MAXB, 1))
    ones16 = np.ones((128, 1), dtype=np.float16)
    ones32 = np.ones((128, 1), dtype=np.float32)
    gb_in = np.concatenate([np.asarray(gamma, np.float32), np.asarray(beta, np.float32)])[None, :]  # [1,512]
    b2rep = np.tile(np.asarray(b2, np.float32)[None, :], (128, 1))  # [128, 64]

    ins = []
    for c in range(R):
        xT = np.ascontiguousarray(x[inv[c * NS:(c + 1) * NS]].T)  # [128, 6250]
        ins.append({
            "xT": xT,
            "W1ext": W1ext,
            "W2ext": W2ext,
            "iota24": iota24.reshape(128, MAXB * 128),
            "ones16": ones16,
            "ones32": ones32,
            "gb_in": gb_in,
            "b2rep": b2rep,
            "idx": packs[c]["idx"],
            "dstpp": packs[c]["dstpp"],
            "eclamp": packs[c]["eclamp"],
            "maskT": packs[c]["maskT"],
        })
    return sched, nblk, ncalls, ins, q


NSP = NT * 128  # padded shard rows (6272)


def build_program(sched, nblk, ncalls):
    nc = bacc.Bacc("TRN2", target_bir_lowering=False, debug=False,
                   num_devices=R, num_swdge_queues=NQ,
                   dynamic_dma_scratch_size=16384)
    xT_d = nc.dram_tensor("xT", [F_IN, NS], F32, kind="ExternalInput")
    W1e_d = nc.dram_tensor("W1ext", [F_IN, HC + 2 * HEADS], F32, kind="ExternalInput")
    W2e_d = nc.dram_tensor("W2ext", [HC, W2C], F16, kind="ExternalInput")
    iota24_d = nc.dram_tensor("iota24", [128, MAXB * 128], F16, kind="ExternalInput")
    ones16_d = nc.dram_tensor("ones16", [128, 1], F16, kind="ExternalInput")
    ones32_d = nc.dram_tensor("ones32", [128, 1], F32, kind="ExternalInput")
    gbin_d = nc.dram_tensor("gb_in", [1, 2 * HC], F32, kind="ExternalInput")
    b2rep_d = nc.dram_tensor("b2rep", [128, CLASSES], F32, kind="ExternalInput")
    idx_d = nc.dram_tensor("idx", [128, nblk * 8], I16, kind="ExternalInput")
    dstpp_d = nc.dram_tensor("dstpp", [128, nblk], F16, kind="ExternalInput")
    eclamp_d = nc.dram_tensor("eclamp", [128, nblk], F16, kind="ExternalInput")
    maskT_d = nc.dram_tensor("maskT", [128, nblk, 128], F16, kind="ExternalInput")
    out_d = nc.dram_tensor("out", [NS, CLASSES], F32, kind="ExternalOutput")

    with tile.TileContext(nc) as tc:
        import contextlib
        ctx = contextlib.ExitStack()
        with ctx:
            cons = ctx.enter_context(tc.tile_pool(name="cons", bufs=1))
            dram = ctx.enter_context(tc.tile_pool(name="dram", bufs=1, space="DRAM"))
            rowp = ctx.enter_context(tc.tile_pool(name="rowp", bufs=2))
            gp = ctx.enter_context(tc.tile_pool(name="gp", bufs=2))
            mp = ctx.enter_context(tc.tile_pool(name="mp", bufs=4))
            mtp = ctx.enter_context(tc.tile_pool(name="mtp", bufs=2))
            ep = ctx.enter_context(tc.tile_pool(name="ep", bufs=2))
            gop = ctx.enter_context(tc.tile_pool(name="gop", bufs=2))
            yp = ctx.enter_context(tc.tile_pool(name="yp", bufs=2))
            zlp = ctx.enter_context(tc.tile_pool(name="zlp", bufs=2))
            sfp = ctx.enter_context(tc.tile_pool(name="sfp", bufs=2))
            ps = ctx.enter_context(tc.tile_pool(name="ps", bufs=2, space="PSUM"))
            ps1 = ctx.enter_context(tc.tile_pool(name="ps1", bufs=1, space="PSUM"))
            bnp = ctx.enter_context(tc.tile_pool(name="bnp", bufs=1, space="PSUM"))

            # ---- constants into SBUF
            xT = cons.tile([F_IN, NS], F32)
            nc.sync.dma_start(out=xT[:], in_=xT_d[:, :])
            W1e = cons.tile([F_IN, HC + 2 * HEADS], F32)
            nc.sync.dma_start(out=W1e[:], in_=W1e_d[:, :])
            W2e0 = cons.tile([128, W2C], F16)
            nc.sync.dma_start(out=W2e0[:], in_=W2e_d[0:128, :])
            W2e1 = cons.tile([128, W2C], F16)
            nc.sync.dma_start(out=W2e1[:], in_=W2e_d[128:256, :])
            iota24 = cons.tile([128, MAXB, 128], F16)
            nc.sync.dma_start(out=iota24[:].rearrange("p a b -> p (a b)"),
                              in_=iota24_d[:, :])
            ones16 = cons.tile([128, 1], F16)
            nc.sync.dma_start(out=ones16[:], in_=ones16_d[:, :])
            ones32 = cons.tile([128, 1], F32)
            nc.sync.dma_start(out=ones32[:], in_=ones32_d[:, :])
            gbrow = cons.tile([1, 2 * HC], F32)
            nc.sync.dma_start(out=gbrow[:], in_=gbin_d[:, :])
            b2rep = cons.tile([128, CLASSES], F32)
            nc.sync.dma_start(out=b2rep[:], in_=b2rep_d[:, :])
            idx_t = cons.tile([128, nblk * 8], I16)
            nc.sync.dma_start(out=idx_t[:], in_=idx_d[:, :])
            dstpp = cons.tile([128, nblk], F16)
            nc.sync.dma_start(out=dstpp[:], in_=dstpp_d[:, :])
            eclamp = cons.tile([128, nblk], F16)
            nc.sync.dma_start(out=eclamp[:], in_=eclamp_d[:, :])
            ad_sh = cons.tile([128, NT, HEADS], F32)
            ad2_sh = cons.tile([128, NT, 1], F32)
            as_sh = cons.tile([128, NT, HEADS], F32)
            as2_sh = cons.tile([128, NT, 1], F32)
            nc.vector.memset(ad_sh[:], 0.0)
            nc.vector.memset(ad2_sh[:], 0.0)
            nc.vector.memset(as_sh[:], 0.0)
            nc.vector.memset(as2_sh[:], 0.0)

            # ---- internal DRAM
            tb1_sh = dram.tile([NS, ROW1], F16)
            tb1 = dram.tile([N, ROW1], F16, addr_space="Shared")
            g_sh = dram.tile([NSP, HC], F16)
            ar_in = dram.tile([1, 2 * HC], F32)
            ar_out = dram.tile([1, 2 * HC], F32, addr_space="Shared")
            gb_d = dram.tile([1, 2 * HC], F32)
            tb2_sh = dram.tile([NS, ROW2], F16)
            tb2 = dram.tile([N, ROW2], F16, addr_space="Shared")

            # zero-init the gather row buffers (slots skipped by the
            # negative-index gather must hold benign values, not NaN bits)
            for _ in range(4):
                rz = gp.tile([128, MAXB, ROW1], F16, tag="rows", bufs=4)
                nc.vector.memset(rz[:], 0.0)

            # ================= Phase A: z1/as1/ad1 tables =================
            for t in range(NT):
                nt = _tile_nodes(t)
                z1 = ps.tile([128, HC + 2 * HEADS], F32, tag="zps")
                nc.tensor.matmul(z1[0:nt, :], lhsT=xT[:, t * 128:t * 128 + nt],
                                 rhs=W1e[:], start=True, stop=True)
                row = rowp.tile([128, ROW1], F16, tag="row1")
                nc.vector.tensor_copy(out=row[0:nt, 0:HC], in_=z1[0:nt, 0:HC])
                nc.scalar.copy(
                    out=row[:].bitcast(F32)[0:nt, HC // 2:HC // 2 + HEADS],
                    in_=z1[0:nt, HC:HC + HEADS])
                nc.scalar.copy(out=ad_sh[0:nt, t, :],
                               in_=z1[0:nt, HC + HEADS:HC + 2 * HEADS])
                nc.scalar.copy(out=as_sh[0:nt, t, :], in_=z1[0:nt, HC:HC + HEADS])
                nc.sync.dma_start(out=tb1_sh[t * 128:t * 128 + nt, :], in_=row[0:nt, :])

            nc.gpsimd.collective_compute(
                "AllGather", OP.bypass, replica_groups=[list(range(R))],
                ins=[tb1_sh.opt()], outs=[tb1.opt()])

            # ================= L1 edge pass =================
            ad16_all = cons.tile([128, NT, HEADS], F16)
            nc.scalar.copy(out=ad16_all[:], in_=ad_sh[:])
            bn = bnp.tile([1, 2 * HC], F32, tag="bn")
            PRE = 2
            l1f = {}

            def l1_front(S):
                t, b0, nb = S["t"], S["blk0"], S["nb"]
                mT = mtp.tile([128, MAXB, 128], F16, tag="mT", bufs=4)
                nc.sync.dma_start(out=mT[:, 0:nb, :], in_=maskT_d[:, b0:b0 + nb, :])
                mE = mtp.tile([128, MAXB, 128], F16, tag="mE", bufs=4)
                nc.vector.tensor_tensor(
                    out=mE[:, 0:nb, :],
                    in0=dstpp[:, b0:b0 + nb].to_broadcast([128, nb, 128]),
                    in1=iota24[:, 0:nb, :],
                    op=OP.is_equal)
                rows = gp.tile([128, MAXB, ROW1], F16, tag="rows", bufs=4)
                for (sec, boff, nbc, ci) in S["calls"]:
                    ni = nbc * 128
                    base = sec * SECT
                    nc.gpsimd.dma_gather(
                        rows[:, boff:boff + nbc, :], tb1[base:base + SECT, :],
                        idx_t[:, (b0 + boff) * 8:(b0 + boff) * 8 + nbc * 8],
                        ni, S["nv"][ci], ROW1, queue_num=ci % NQ)
                ad_ps = ps1.tile([128, MAXB * HEADS], F32, tag="adps", bufs=2)
                for b in range(nb):
                    nc.tensor.matmul(ad_ps[:, b * HEADS:(b + 1) * HEADS],
                                     lhsT=mT[:, b, :], rhs=ad16_all[:, t, :],
                                     start=True, stop=True)
                z1loc = zlp.tile([128, HC], F16, tag="z1loc", bufs=3)
                nt = _tile_nodes(t)
                nc.sync.dma_start(out=z1loc[0:nt, :], in_=tb1_sh[t * 128:t * 128 + nt, 0:HC])
                l1f[t] = (mE, rows, ad_ps, z1loc)

            for tt in range(min(PRE, NT)):
                l1_front(sched[tt])
            for ti, S in enumerate(sched):
                if ti + PRE < NT:
                    l1_front(sched[ti + PRE])
                t, b0, nb = S["t"], S["blk0"], S["nb"]
                nt = _tile_nodes(t)
                mE, rows, ad_ps, z1loc = l1f.pop(t)
                out_ps = ps.tile([128, HC + HEADS], F32, tag="outps", bufs=3)
                e_t = ep.tile([128, MAXB, HEADS], F32, tag="e")
                nc.vector.tensor_tensor(
                    out=e_t[:, 0:nb, :],
                    in0=rows[:].bitcast(F32)[:, 0:nb, HC // 2:HC // 2 + HEADS],
                    in1=ad_ps[:, 0:nb * HEADS].rearrange("p (b h) -> p b h", h=HEADS),
                    op=OP.add)
                e2_t = ep.tile([128, MAXB, HEADS], F32, tag="e2")
                nc.vector.tensor_scalar(out=e2_t[:, 0:nb, :], in0=e_t[:, 0:nb, :],
                                        scalar1=NEG_ATT, scalar2=None, op0=OP.mult)
                nc.vector.tensor_tensor(out=e_t[:, 0:nb, :], in0=e_t[:, 0:nb, :],
                                        in1=e2_t[:, 0:nb, :], op=OP.max)
                nc.vector.tensor_tensor(out=e_t[:, 0:nb, :], in0=e_t[:, 0:nb, :],
                                        in1=eclamp[:, b0:b0 + nb].to_broadcast([128, nb, HEADS]),
                                        op=OP.min)
                nc.scalar.activation(rows[:, 0:nb, HC:HC + HEADS],
                                     e_t[:, 0:nb, :], ACTF.Exp)
                nc.vector.tensor_tensor(
                    out=rows[:, 0:nb, 0:HC].rearrange("p b (h c) -> p b h c", h=HEADS),
                    in0=rows[:, 0:nb, 0:HC].rearrange("p b (h c) -> p b h c", h=HEADS),
                    in1=rows[:, 0:nb, HC:HC + HEADS].to_broadcast([128, nb, HEADS, HID]),
                    op=OP.mult)
                for b in range(nb):
                    nc.tensor.matmul(out_ps[:, :], lhsT=mE[:, b, :],
                                     rhs=rows[:, b, 0:HC + HEADS],
                                     start=(b == 0), stop=(b == nb - 1))
                es = ep.tile([128, HEADS], F32, tag="es")
                nc.vector.tensor_tensor(out=es[:], in0=as_sh[:, t, :],
                                        in1=ad_sh[:, t, :], op=OP.add)
                es2 = ep.tile([128, HEADS], F32, tag="es2")
                nc.vector.tensor_scalar(out=es2[:], in0=es[:], scalar1=NEG_ATT,
                                        scalar2=None, op0=OP.mult)
                nc.vector.tensor_tensor(out=es[:], in0=es[:], in1=es2[:], op=OP.max)
                psf = ep.tile([128, HEADS], F32, tag="psf")
                nc.scalar.activation(psf[:], es[:], ACTF.Exp)
                dtm = ep.tile([128, HEADS], F32, tag="dtm")
                nc.vector.tensor_tensor(out=dtm[:], in0=out_ps[:, HC:HC + HEADS],
                                        in1=psf[:], op=OP.add)
                dre = ep.tile([128, HEADS], F32, tag="dre")
                nc.vector.reciprocal(out=dre[:], in_=dtm[:])
                sfw = sfp.tile([128, HC], F32, tag="sfw")
                nc.vector.tensor_tensor(
                    out=sfw[:].rearrange("p (h c) -> p h c", h=HEADS),
                    in0=z1loc[:].rearrange("p (h c) -> p h c", h=HEADS),
                    in1=psf[:].to_broadcast([128, HEADS, HID]),
                    op=OP.mult)
                nc.vector.tensor_tensor(out=sfw[:], in0=sfw[:],
                                        in1=out_ps[:, 0:HC], op=OP.add)
                gsq = gop.tile([128, 2 * HC], F16, tag="gsq")
                nc.vector.tensor_tensor(
                    out=gsq[:, 0:HC].rearrange("p (h c) -> p h c", h=HEADS),
                    in0=sfw[:].rearrange("p (h c) -> p h c", h=HEADS),
                    in1=dre[:].to_broadcast([128, HEADS, HID]),
                    op=OP.mult)
                nc.vector.tensor_tensor(out=gsq[0:nt, HC:2 * HC], in0=gsq[0:nt, 0:HC],
                                        in1=gsq[0:nt, 0:HC], op=OP.mult)
                nc.tensor.matmul(bn[:, :], lhsT=ones16[0:nt, :], rhs=gsq[0:nt, :],
                                 start=(ti == 0), stop=(ti == NT - 1))
                nc.sync.dma_start(out=g_sh[t * 128:t * 128 + nt, :], in_=gsq[0:nt, 0:HC])

            # zero the padded tail rows of g_sh
            zr = gop.tile([128, 2 * HC], F16, tag="gsq")
            nc.vector.memset(zr[:], 0.0)
            if NSP > NS:
                nc.sync.dma_start(out=g_sh[NS:NSP, :], in_=zr[0:NSP - NS, 0:HC])

            # ================= BN stats: AllReduce + affine =================
            bnst = cons.tile([1, 2 * HC], F32)
            nc.vector.tensor_copy(out=bnst[:], in_=bn[:, :])
            nc.sync.dma_start(out=ar_in[:], in_=bnst[:])
            nc.gpsimd.collective_compute(
                "AllReduce", OP.add, replica_groups=[list(range(R))],
                ins=[ar_in.opt()], outs=[ar_out.opt()])
            st = cons.tile([1, 2 * HC], F32)
            nc.sync.dma_start(out=st[:], in_=ar_out[:])
            mu = cons.tile([1, HC], F32)
            nc.vector.tensor_scalar(out=mu[:], in0=st[:, 0:HC], scalar1=1.0 / N,
                                    scalar2=None, op0=OP.mult)
            var = cons.tile([1, HC], F32)
            nc.vector.tensor_scalar(out=var[:], in0=st[:, HC:2 * HC], scalar1=1.0 / N,
                                    scalar2=None, op0=OP.mult)
            musq = cons.tile([1, HC], F32)
            nc.vector.tensor_tensor(out=musq[:], in0=mu[:], in1=mu[:], op=OP.mult)
            nc.vector.tensor_tensor(out=var[:], in0=var[:], in1=musq[:], op=OP.subtract)
            nc.vector.tensor_scalar(out=var[:], in0=var[:], scalar1=BN_EPS, scalar2=None,
                                    op0=OP.add)
            rv = cons.tile([1, HC], F32)
            nc.vector.reciprocal(out=rv[:], in_=var[:])
            rs = cons.tile([1, HC], F32)
            nc.scalar.activation(rs[:], rv[:], ACTF.Sqrt)
            gp_ = cons.tile([1, HC], F32)
            nc.vector.tensor_tensor(out=gp_[:], in0=rs[:], in1=gbrow[:, 0:HC], op=OP.mult)
            bp_ = cons.tile([1, HC], F32)
            nc.vector.tensor_tensor(out=bp_[:], in0=gp_[:], in1=mu[:], op=OP.mult)
            nc.vector.tensor_tensor(out=bp_[:], in0=gbrow[:, HC:2 * HC], in1=bp_[:],
                                    op=OP.subtract)
            nc.sync.dma_start(out=gb_d[0:1, 0:HC], in_=gp_[:])
            nc.sync.dma_start(out=gb_d[0:1, HC:2 * HC], in_=bp_[:])

            # read gamma'/beta' transposed: [128,1] per feature half
            gb_cols = gb_d.rearrange("a (f x) -> (a f) x", x=1)
            gpp = [cons.tile([128, 1], F32, name=f"gpp{i}") for i in range(2)]
            bpp = [cons.tile([128, 1], F32, name=f"bpp{i}") for i in range(2)]
            for h in range(2):
                nc.sync.dma_start(out=gpp[h][:], in_=gb_cols[h * 128:(h + 1) * 128, :])
                nc.sync.dma_start(out=bpp[h][:], in_=gb_cols[HC + h * 128:HC + (h + 1) * 128, :])

            # ================= P4: y (transposed) and z2 table =================
            chunks = [(i * 512, 512) for i in range(NSP // 512)]
            if NSP % 512:
                chunks.append((NSP - NSP % 512, NSP % 512))
            for (n0, w) in chunks:
                yT = []
                for h in range(2):
                    gT = yp.tile([128, 512], F16, tag=f"gT{h}")
                    nc.sync.dma_start(out=gT[:, 0:w],
                                      in_=g_sh[n0:n0 + w, h * 128:(h + 1) * 128],
                                      transpose=True)
                    nc.vector.tensor_scalar(out=gT[:, 0:w], in0=gT[:, 0:w],
                                            scalar1=gpp[h][:, :], scalar2=bpp[h][:, :],
                                            op0=OP.mult, op1=OP.add)
                    sm = yp.tile([128, 512], F16, tag=f"sm{h}")
                    nc.vector.tensor_scalar(out=sm[:, 0:w], in0=gT[:, 0:w],
                                            scalar1=NEG_ACT, scalar2=None, op0=OP.mult)
                    nc.vector.tensor_tensor(out=gT[:, 0:w], in0=gT[:, 0:w],
                                            in1=sm[:, 0:w], op=OP.max)
                    yT.append(gT)
                for i in range(w // 128):
                    t = (n0 + i * 128) // 128
                    nt = _tile_nodes(t) if t < NT else 0
                    if nt == 0:
                        continue
                    z2 = ps.tile([128, W2C], F32, tag="zps")
                    nc.tensor.matmul(z2[:, :], lhsT=yT[0][:, i * 128:(i + 1) * 128],
                                     rhs=W2e0[:], start=True, stop=False)
                    nc.tensor.matmul(z2[:, :], lhsT=yT[1][:, i * 128:(i + 1) * 128],
                                     rhs=W2e1[:], start=False, stop=True)
                    row2 = rowp.tile([128, ROW2], F16, tag="row2")
                    nc.scalar.copy(out=row2[0:nt, 0:CLASSES], in_=z2[0:nt, 0:CLASSES])
                    nc.scalar.copy(
                        out=row2[:].bitcast(F32)[0:nt, CLASSES // 2:CLASSES // 2 + 1],
                        in_=z2[0:nt, CLASSES:CLASSES + 1])
                    nc.scalar.copy(out=ad2_sh[0:nt, t, :],
                                   in_=z2[0:nt, CLASSES + 1:CLASSES + 2])
                    nc.scalar.copy(out=as2_sh[0:nt, t, :],
                                   in_=z2[0:nt, CLASSES:CLASSES + 1])
                    nc.sync.dma_start(out=tb2_sh[t * 128:t * 128 + nt, :], in_=row2[0:nt, :])

            nc.gpsimd.collective_compute(
                "AllGather", OP.bypass, replica_groups=[list(range(R))],
                ins=[tb2_sh.opt()], outs=[tb2.opt()])

            # ================= L2 edge pass =================
            # re-zero the gather buffers: stale L1-format bytes reinterpreted
            # at L2 offsets can alias to f16 NaN patterns (0 * NaN = NaN)
            for _ in range(4):
                rz2 = gp.tile([128, MAXB, ROW1], F16, tag="rows", bufs=4)
                nc.vector.memset(rz2[:], 0.0)
            ad2_16 = cons.tile([128, NT, 1], F16)
            nc.scalar.copy(out=ad2_16[:], in_=ad2_sh[:])
            l2f = {}

            def l2_front(S):
                t, b0, nb = S["t"], S["blk0"], S["nb"]
                mT = mtp.tile([128, MAXB, 128], F16, tag="mT", bufs=4)
                nc.sync.dma_start(out=mT[:, 0:nb, :], in_=maskT_d[:, b0:b0 + nb, :])
                mE = mtp.tile([128, MAXB, 128], F16, tag="mE", bufs=4)
                nc.vector.tensor_tensor(
                    out=mE[:, 0:nb, :],
                    in0=dstpp[:, b0:b0 + nb].to_broadcast([128, nb, 128]),
                    in1=iota24[:, 0:nb, :],
                    op=OP.is_equal)
                rows = gp.tile([128, MAXB, ROW2], F16, tag="rows", bufs=4)
                for (sec, boff, nbc, ci) in S["calls"]:
                    ni = nbc * 128
                    base = sec * SECT
                    nc.gpsimd.dma_gather(
                        rows[:, boff:boff + nbc, :], tb2[base:base + SECT, :],
                        idx_t[:, (b0 + boff) * 8:(b0 + boff) * 8 + nbc * 8],
                        ni, S["nv"][ci], ROW2, queue_num=ci % NQ)
                ad_ps = ps1.tile([128, MAXB * HEADS], F32, tag="adps", bufs=2)
                for b in range(nb):
                    nc.tensor.matmul(ad_ps[:, b:b + 1],
                                     lhsT=mT[:, b, :], rhs=ad2_16[:, t, :],
                                     start=True, stop=True)
                z2loc = zlp.tile([128, CLASSES], F16, tag="z2loc", bufs=3)
                nt = _tile_nodes(t)
                nc.sync.dma_start(out=z2loc[0:nt, :],
                                  in_=tb2_sh[t * 128:t * 128 + nt, 0:CLASSES])
                l2f[t] = (mE, rows, ad_ps, z2loc)

            for tt in range(min(PRE, NT)):
                l2_front(sched[tt])
            for ti, S in enumerate(sched):
                if ti + PRE < NT:
                    l2_front(sched[ti + PRE])
                t, b0, nb = S["t"], S["blk0"], S["nb"]
                nt = _tile_nodes(t)
                mE, rows, ad_ps, z2loc = l2f.pop(t)
                out_ps = ps.tile([128, HC + HEADS], F32, tag="outps", bufs=3)
                e_t = ep.tile([128, MAXB, HEADS], F32, tag="e")
                nc.vector.tensor_tensor(
                    out=e_t[:, 0:nb, 0:1],
                    in0=rows[:].bitcast(F32)[:, 0:nb, CLASSES // 2:CLASSES // 2 + 1],
                    in1=ad_ps[:, 0:nb].rearrange("p (b h) -> p b h", h=1),
                    op=OP.add)
                e2_t = ep.tile([128, MAXB, HEADS], F32, tag="e2")
                nc.vector.tensor_scalar(out=e2_t[:, 0:nb, 0:1], in0=e_t[:, 0:nb, 0:1],
                                        scalar1=NEG_ATT, scalar2=None, op0=OP.mult)
                nc.vector.tensor_tensor(out=e_t[:, 0:nb, 0:1], in0=e_t[:, 0:nb, 0:1],
                                        in1=e2_t[:, 0:nb, 0:1], op=OP.max)
                nc.vector.tensor_tensor(out=e_t[:, 0:nb, 0:1], in0=e_t[:, 0:nb, 0:1],
                                        in1=eclamp[:, b0:b0 + nb].to_broadcast([128, nb, 1]),
                                        op=OP.min)
                nc.scalar.activation(rows[:, 0:nb, CLASSES:CLASSES + 1],
                                     e_t[:, 0:nb, 0:1], ACTF.Exp)
                nc.vector.tensor_tensor(
                    out=rows[:, 0:nb, 0:CLASSES],
                    in0=rows[:, 0:nb, 0:CLASSES],
                    in1=rows[:, 0:nb, CLASSES:CLASSES + 1].to_broadcast([128, nb, CLASSES]),
                    op=OP.mult)
                for b in range(nb):
                    nc.tensor.matmul(out_ps[:, 0:CLASSES + 1], lhsT=mE[:, b, :],
                                     rhs=rows[:, b, 0:CLASSES + 1],
                                     start=(b == 0), stop=(b == nb - 1))
                es = ep.tile([128, HEADS], F32, tag="es")
                nc.vector.tensor_tensor(out=es[:, 0:1], in0=as2_sh[:, t, :],
                                        in1=ad2_sh[:, t, :], op=OP.add)
                es2 = ep.tile([128, HEADS], F32, tag="es2")
                nc.vector.tensor_scalar(out=es2[:, 0:1], in0=es[:, 0:1], scalar1=NEG_ATT,
                                        scalar2=None, op0=OP.mult)
                nc.vector.tensor_tensor(out=es[:, 0:1], in0=es[:, 0:1], in1=es2[:, 0:1],
                                        op=OP.max)
                psf = ep.tile([128, HEADS], F32, tag="psf")
                nc.scalar.activation(psf[:, 0:1], es[:, 0:1], ACTF.Exp)
                dtm = ep.tile([128, HEADS], F32, tag="dtm")
                nc.vector.tensor_tensor(out=dtm[:, 0:1], in0=out_ps[:, CLASSES:CLASSES + 1],
                                        in1=psf[:, 0:1], op=OP.add)
                dre = ep.tile([128, 1], F32, tag="dre2")
                nc.vector.reciprocal(out=dre[:], in_=dtm[:, 0:1])
                sfw = sfp.tile([128, HC], F32, tag="sfw")
                nc.vector.tensor_tensor(out=sfw[:, 0:CLASSES], in0=z2loc[:],
                                        in1=psf[:, 0:1].to_broadcast([128, CLASSES]),
                                        op=OP.mult)
                nc.vector.tensor_tensor(out=sfw[:, 0:CLASSES], in0=sfw[:, 0:CLASSES],
                                        in1=out_ps[:, 0:CLASSES], op=OP.add)
                o32 = gop.tile([128, CLASSES], F32, tag="o32")
                nc.vector.tensor_scalar(out=o32[:], in0=sfw[:, 0:CLASSES], scalar1=dre[:, :],
                                        scalar2=None, op0=OP.mult)
                nc.vector.tensor_tensor(out=o32[:], in0=o32[:], in1=b2rep[:], op=OP.add)
                nc.sync.dma_start(out=out_d[t * 128:t * 128 + nt, :], in_=o32[0:nt, :])
    nc.compile()
    return nc


def kernel(x, edge_index, W1, a_src1, a_dst1, b1, gamma, beta, W2, a_src2, a_dst2, b2):
    # b1 cancels inside BatchNorm (constant per-channel shift) -> unused.
    sched, nblk, ncalls, ins, q = host_inputs(np.asarray(x), np.asarray(edge_index),
                                              W1, a_src1, a_dst1, gamma, beta,
                                              W2, a_src2, a_dst2, b2)
    nc = build_program(sched, nblk, ncalls)
    res = bass_utils.run_bass_kernel_spmd(nc, ins, core_ids=list(range(R)))
    out = np.concatenate([res.results[c]["out"] for c in range(R)], axis=0)
    return out[q].astype(np.float32)
